# revision 35
# baseline (speedup 1.0000x reference)
"""Trainium2 Bass kernel for nn_ConvolutionalAttention_3015067042131.

Math (reference.py):
  x [16,128,64,64] f32; x1 = x[:, :64], x2 = x[:, 64:]
  pooled = mean(x1, HW); h = gelu(pooled @ w1.T + b1); dyn = (h @ w2.T + b2) -> [B,64,9]
  x1_dyn = per-(batch,channel) 3x3 depthwise conv of x1 with dyn
  x1_lk  = conv2d(x1, lk_filter[64,64,13,13], SAME)
  out = concat([x1_lk + x1_dyn, x2], ch)

Strategy (v2):
  * Tiny MLP (dyn) on host in float64; dynamic 3x3 folded into the 13x13
    weights as per-batch diagonal additions on the central taps.
  * Conv as shift-and-matmul, taps packed two-per-matmul along K=128:
    - 78 horizontal pairs (i, 2p)+(i, 2p+1): SBUF partitions 0-63 hold the
      zero-padded image, 64-127 hold it shifted LEFT one column (layout LA).
    - 6 vertical pairs (2v,12)+(2v+1,12): partitions 64-127 hold the image
      shifted UP one row (layout LB).  - 1 single (12,12).
    85 tap-tiles cover all 169 taps (vs 91 in v1).
  * Both padded layouts are built on HOST and DMA'd contiguously
    (11.5KB/partition runs) -- zero on-chip layout work, so the first
    matmul is gated only by a ~0.7MB DMA prefix (interleaved row/tile
    DMA slices; the single DMA queue completes roughly FIFO).
  * Per tap tile: two matmuls (tile_position (0,0)/(0,64), PSUM
    partitions 0-63/64-127 = two 512-pixel chunks) stream concurrently
    in the two PE column halves; per-MM LDWEIGHTS issue (~107ns) stays
    just under the 213ns N=512 stream, slot cadence ~219ns.
    (Tried and rejected: one shared LDWEIGHTS per tile via IR surgery --
    the PE's background weight buffer corrupts under queue backup and
    serializes with --enable-ldw-opt=false; N=1024 matmuls -- ISA
    rejects >512 output elements; fp8 -- quantization error ~4e-2
    exceeds the 2e-2 gate at this reduction size.)
  * Short-N warmup junk matmuls bridge the framework preamble to
    first-data so the HAM clock gate never re-throttles (v1 lost ~19us
    to a 9.3us PE idle + 1.2GHz cold restart mid-kernel).
  * fp16 output (host upcasts); fp16 operand numerics: end-to-end rel
    err ~4e-4 vs the 2e-2 gate.
  * Sharding: data-parallel over batch, 2 images per core on 8 cores.
    x2 passthrough is host-side.
"""
import math

import numpy as np

B, C, H, W = 16, 128, 64, 64
PDIM, SK, LK = 64, 3, 13
PAD = LK // 2  # 6
HP, WP = H + 2 * PAD, W + 2 * PAD  # 76, 76
NCORES = 8
BPC = B // NCORES  # images per core
NT = 85            # tap tiles: 78 horizontal pairs + 6 vertical pairs + 1 single
NCHUNK = 8         # 512-pixel chunks per image
CHUNK = H * W // NCHUNK  # 512
NWARM = 38         # junk N=128 matmuls bridging preamble -> first data

# tile t: t = i*6+p (p<6) -> horizontal pair ((i,2p),(i,2p+1));
#         t = 78+v -> vertical pair ((2v,12),(2v+1,12)); t = 84 -> single (12,12)
# central 3x3 dyn taps (i,j in 5..7): j=5 -> tile i*6+2 (B half), j=6/7 -> tile
# i*6+3 (A/B halves); 6 per-batch modified tiles.
_MOD_TILES = [5 * 6 + 2, 6 * 6 + 2, 7 * 6 + 2, 5 * 6 + 3, 6 * 6 + 3, 7 * 6 + 3]
_MOD_SLOT = {t: s for s, t in enumerate(_MOD_TILES)}

_ERF = np.vectorize(math.erf, otypes=[np.float64])

_CACHED_NC = None


def _build_nc():
    import concourse.mybir as mybir
    import concourse.tile as tile
    from concourse import bacc

    f32 = mybir.dt.float32
    f16 = mybir.dt.float16

    nc = bacc.Bacc(None, target_bir_lowering=False)
    la = nc.dram_tensor("la", [BPC, 128, HP, WP], f16, kind="ExternalInput")
    lb = nc.dram_tensor("lb", [BPC, 128, HP, WP], f16, kind="ExternalInput")
    wsh = nc.dram_tensor("wsh", [128, NT * 80], f16, kind="ExternalInput")
    wmod = nc.dram_tensor("wmod", [BPC, 128, 6 * 64], f16, kind="ExternalInput")
    y = nc.dram_tensor("y", [BPC, PDIM, H * W], f16, kind="ExternalOutput")

    NSH_FIRST = 28  # weight tiles shipped in the leading DMA slice

    with tile.TileContext(nc) as tc:
        with (
            tc.tile_pool(name="wpool", bufs=1) as wpool,
            tc.tile_pool(name="wmpool", bufs=2) as wmpool,
            tc.tile_pool(name="xpool", bufs=2) as xpool,
            tc.tile_pool(name="xbpool", bufs=2) as xbpool,
            tc.tile_pool(name="opool", bufs=3) as opool,
            tc.tile_pool(name="pspool", bufs=4, space="PSUM") as pspool,
        ):
            # input DMAs, in rough order of need; row-split so chunk-pair 0
            # can start after the first slice (tile deps are region-overlap)
            la_sb = [xpool.tile([128, HP, WP], f16, name=f"la{b}") for b in range(BPC)]
            lb_sb = [xbpool.tile([128, HP, WP], f16, name=f"lb{b}") for b in range(BPC)]
            wsh_sb = wpool.tile([128, NT * 80], f16)
            wm_sb = [wmpool.tile([128, 6 * 64], f16, name=f"wm{b}") for b in range(BPC)]

            # the single DMA queue drains roughly FIFO at ~360GB/s after a
            # ~2us posting ramp, so completion time ~= cumulative prefix
            # bytes; order by first use.  chunk-pair cp of tap row i reads
            # la rows i+16cp .. i+16cp+16.
            nc.sync.dma_start(out=la_sb[0][:, 0:16, :], in_=la[0, :, 0:16, :])
            nc.sync.dma_start(out=wsh_sb[:, 0:480], in_=wsh[:, 0:480])
            nc.sync.dma_start(out=la_sb[0][:, 16:24, :], in_=la[0, :, 16:24, :])
            nc.sync.dma_start(out=wsh_sb[:, 480:1600], in_=wsh[:, 480:1600])
            nc.sync.dma_start(out=wm_sb[0][:], in_=wmod[0])
            nc.sync.dma_start(out=wsh_sb[:, 1600:2720], in_=wsh[:, 1600:2720])
            nc.sync.dma_start(out=la_sb[0][:, 24:40, :], in_=la[0, :, 24:40, :])
            nc.sync.dma_start(out=wsh_sb[:, 2720:4160], in_=wsh[:, 2720:4160])
            nc.sync.dma_start(out=la_sb[0][:, 40:, :], in_=la[0, :, 40:, :])
            nc.sync.dma_start(out=wsh_sb[:, 4160:], in_=wsh[:, 4160:])
            nc.sync.dma_start(out=lb_sb[0][:], in_=lb[0])
            nc.sync.dma_start(out=la_sb[1][:], in_=la[1])
            nc.sync.dma_start(out=lb_sb[1][:], in_=lb[1])
            # wm1 rides the scalar hwdge queue: content isn't needed until
            # ~85us, and the post spins up that queue (~4.6us ramp) so the
            # final output DMA can use it concurrently with the sync queue
            nc.scalar.dma_start(out=wm_sb[1][:], in_=wmod[1])

            # PE warmup: junk matmuls on a zeroed scratch tile keep the PE busy
            # (HAM un-throttles 1.2 -> 2.4 GHz) while the input DMAs run.
            # short-N junk matmuls: fine-grained bridge (one MM ~107ns cold /
            # 56ns warm) so the in-order PE queue frees up right when the
            # first data lands
            scratch = wpool.tile([128, 128], f16)
            nc.vector.memset(scratch[:], 0.0)
            ps_warm = pspool.tile([128, 128], f32, name="ps_warm", bufs=1)
            for wi in range(NWARM):
                nc.tensor.matmul(
                    ps_warm[0:64, :],
                    lhsT=scratch[:, 0:64],
                    rhs=scratch[:, :],
                    start=(wi == 0),
                    stop=(wi == NWARM - 1),
                    tile_position=(0, 0),
                    skip_group_check=True,
                )

            for b in range(BPC):
                for cp in range(NCHUNK // 2):
                    ps = pspool.tile([128, CHUNK], f32)
                    for t in range(NT):
                        s = _MOD_SLOT.get(t)
                        w_ap = (
                            wm_sb[b][:, s * 64 : (s + 1) * 64]
                            if s is not None
                            else wsh_sb[:, t * 80 : t * 80 + 64]
                        )
                        for half in (0, 1):
                            R = 8 * (2 * cp + half)
                            if t < 78:
                                i, p = divmod(t, 6)
                                rhs = la_sb[b][:, i + R : i + R + 8, 2 * p : 2 * p + 64]
                            elif t < 84:
                                v = t - 78
                                rhs = lb_sb[b][:, 2 * v + R : 2 * v + R + 8, 12:76]
                            else:
                                rhs = la_sb[b][:, 12 + R : 12 + R + 8, 12:76]
                            nc.tensor.matmul(
                                ps[64 * half : 64 * (half + 1), :],
                                lhsT=w_ap,
                                rhs=rhs,
                                start=(t == 0),
                                stop=(t == NT - 1),
                                tile_position=(0, 64 * half),
                                skip_group_check=True,
                            )
                    ot = opool.tile([128, CHUNK], f16)
                    last = b == BPC - 1 and cp == NCHUNK // 2 - 1
                    if last:
                        # final drain: one cast (cost scales with free dim,
                        # not partitions), then ship the halves on BOTH hwdge
                        # queues so posts+transfers overlap
                        nc.vector.tensor_copy(ot[:], ps[:])
                        nc.sync.dma_start(
                            out=y[b, :, (2 * cp) * CHUNK : (2 * cp + 1) * CHUNK],
                            in_=ot[0:64, :],
                        )
                        nc.scalar.dma_start(
                            out=y[b, :, (2 * cp + 1) * CHUNK : (2 * cp + 2) * CHUNK],
                            in_=ot[64:128, :],
                        )
                    else:
                        # drain on the idle Scalar engine: its PSUM reads
                        # don't contend with the PE like DVE's do (DVE drain
                        # was slowing tiles 1-5 of the next chunk-pair)
                        nc.scalar.copy(ot[:], ps[:])
                        nc.sync.dma_start(
                            out=y[b, :, (2 * cp) * CHUNK : (2 * cp + 1) * CHUNK],
                            in_=ot[0:64, :],
                        )
                        nc.sync.dma_start(
                            out=y[b, :, (2 * cp + 1) * CHUNK : (2 * cp + 2) * CHUNK],
                            in_=ot[64:128, :],
                        )
    nc.compile()
    return nc


def _get_nc():
    global _CACHED_NC
    if _CACHED_NC is None:
        _CACHED_NC = _build_nc()
    return _CACHED_NC


def _host_dyn(x, w1, b1, w2, b2):
    """dwc_proj MLP on host, float64: dyn [B, 64, 9]."""
    pooled = x[:, :PDIM].mean(axis=(2, 3), dtype=np.float64)      # [B, 64]
    z = pooled @ w1.T.astype(np.float64) + b1.astype(np.float64)  # [B, 32]
    h = 0.5 * z * (1.0 + _ERF(z / math.sqrt(2.0)))                # exact gelu
    dyn = h @ w2.T.astype(np.float64) + b2.astype(np.float64)     # [B, 576]
    return dyn.reshape(B, PDIM, SK * SK)


def _host_weights(lk_filter, dyn):
    """Build shared supertiles + per-batch modified central supertiles.

    Tap tile t is [128, 64]: rows 0-63 = lk[o, c, tapA].T, rows 64-127 =
    tapB (lhsT layout [K=c, M=o]).  Supertile = the tile duplicated in
    both column halves -> [128, 128]."""
    lkT = lk_filter.transpose(1, 0, 2, 3).astype(np.float32)  # [c, o, i, j]
    Wt = np.zeros((NT, 128, 64), np.float32)
    for i in range(LK):
        for p in range(6):
            Wt[i * 6 + p, 0:64, :] = lkT[:, :, i, 2 * p]
            Wt[i * 6 + p, 64:128, :] = lkT[:, :, i, 2 * p + 1]
    for v in range(6):
        Wt[78 + v, 0:64, :] = lkT[:, :, 2 * v, 12]
        Wt[78 + v, 64:128, :] = lkT[:, :, 2 * v + 1, 12]
    Wt[84, 0:64, :] = lkT[:, :, 12, 12]

    ar = np.arange(64)
    Wmod = np.zeros((B, 6, 128, 64), np.float32)
    for ii, i in enumerate((5, 6, 7)):
        t2, t3 = i * 6 + 2, i * 6 + 3
        u = i - 5
        for b in range(B):
            m2 = Wt[t2].copy()
            m3 = Wt[t3].copy()
            m2[64 + ar, ar] += dyn[b, :, u * 3 + 0].astype(np.float32)  # tap (i,5)
            m3[ar, ar] += dyn[b, :, u * 3 + 1].astype(np.float32)       # tap (i,6)
            m3[64 + ar, ar] += dyn[b, :, u * 3 + 2].astype(np.float32)  # tap (i,7)
            Wmod[b, ii] = m2
            Wmod[b, 3 + ii] = m3

    Wp = np.zeros((NT, 128, 80), np.float32)   # 160B tile stride: rotate
    Wp[:, :, 0:64] = Wt                        # SBUF address phase per tile
    wsh_np = np.ascontiguousarray(
        Wp.transpose(1, 0, 2).reshape(128, NT * 80)
    ).astype(np.float16)
    wmod_np = np.ascontiguousarray(
        Wmod.transpose(0, 2, 1, 3).reshape(B, 128, 6 * 64)
    ).astype(np.float16)
    return wsh_np, wmod_np


def _host_layouts(x1_f16):
    """Padded SBUF layouts, host-built.  LA: partitions 0-63 image at
    (row+6, col+6), 64-127 shifted left one column (col+5).  LB: 0-63
    same, 64-127 shifted up one row (row+5)."""
    la = np.zeros((B, 128, HP, WP), np.float16)
    lb = np.zeros((B, 128, HP, WP), np.float16)
    la[:, 0:64, PAD : PAD + H, PAD : PAD + W] = x1_f16
    la[:, 64:128, PAD : PAD + H, PAD - 1 : PAD - 1 + W] = x1_f16
    lb[:, 0:64, PAD : PAD + H, PAD : PAD + W] = x1_f16
    lb[:, 64:128, PAD - 1 : PAD - 1 + H, PAD : PAD + W] = x1_f16
    return la, lb


def _prepare_in_maps(x, lk_filter, w1, b1, w2, b2):
    x = np.asarray(x, dtype=np.float32)
    dyn = _host_dyn(x, np.asarray(w1), np.asarray(b1), np.asarray(w2), np.asarray(b2))
    wsh_np, wmod_np = _host_weights(np.asarray(lk_filter, dtype=np.float32), dyn)
    x1_f16 = x[:, :PDIM].astype(np.float16)
    la, lb = _host_layouts(x1_f16)
    in_maps = []
    for k in range(NCORES):
        b0 = k * BPC
        in_maps.append(
            {
                "la": np.ascontiguousarray(la[b0 : b0 + BPC]),
                "lb": np.ascontiguousarray(lb[b0 : b0 + BPC]),
                "wsh": wsh_np,
                "wmod": np.ascontiguousarray(wmod_np[b0 : b0 + BPC]),
            }
        )
    return in_maps


def kernel(x, lk_filter, w1, b1, w2, b2):
    from concourse.bass_utils import run_bass_kernel_spmd

    x = np.asarray(x, dtype=np.float32)
    in_maps = _prepare_in_maps(x, lk_filter, w1, b1, w2, b2)
    nc = _get_nc()
    res = run_bass_kernel_spmd(nc, in_maps, core_ids=list(range(NCORES)))

    out = np.empty((B, C, H, W), np.float32)
    for k in range(NCORES):
        b0 = k * BPC
        out[b0 : b0 + BPC, :PDIM] = (
            res.results[k]["y"].astype(np.float32).reshape(BPC, PDIM, H, W)
        )
    out[:, PDIM:] = x[:, PDIM:]
    return out


# revision 41
# speedup vs baseline: 1.0040x; 1.0040x over previous
"""Trainium2 Bass kernel for nn_ConvolutionalAttention_3015067042131.

Math (reference.py):
  x [16,128,64,64] f32; x1 = x[:, :64], x2 = x[:, 64:]
  pooled = mean(x1, HW); h = gelu(pooled @ w1.T + b1); dyn = (h @ w2.T + b2) -> [B,64,9]
  x1_dyn = per-(batch,channel) 3x3 depthwise conv of x1 with dyn
  x1_lk  = conv2d(x1, lk_filter[64,64,13,13], SAME)
  out = concat([x1_lk + x1_dyn, x2], ch)

Strategy (v2):
  * Tiny MLP (dyn) on host in float64; dynamic 3x3 folded into the 13x13
    weights as per-batch diagonal additions on the central taps.
  * Conv as shift-and-matmul, taps packed two-per-matmul along K=128:
    - 78 horizontal pairs (i, 2p)+(i, 2p+1): SBUF partitions 0-63 hold the
      zero-padded image, 64-127 hold it shifted LEFT one column (layout LA).
    - 6 vertical pairs (2v,12)+(2v+1,12): partitions 64-127 hold the image
      shifted UP one row (layout LB).  - 1 single (12,12).
    85 tap-tiles cover all 169 taps (vs 91 in v1).
  * Both padded layouts are built on HOST and DMA'd contiguously
    (11.5KB/partition runs) -- zero on-chip layout work, so the first
    matmul is gated only by a ~0.7MB DMA prefix (interleaved row/tile
    DMA slices; the single DMA queue completes roughly FIFO).
  * Per tap tile: two matmuls (tile_position (0,0)/(0,64), PSUM
    partitions 0-63/64-127 = two 512-pixel chunks) stream concurrently
    in the two PE column halves; per-MM LDWEIGHTS issue (~107ns) stays
    just under the 213ns N=512 stream, slot cadence ~219ns.
    (Tried and rejected: one shared LDWEIGHTS per tile via IR surgery --
    the PE's background weight buffer corrupts under queue backup and
    serializes with --enable-ldw-opt=false; N=1024 matmuls -- ISA
    rejects >512 output elements; fp8 -- quantization error ~4e-2
    exceeds the 2e-2 gate at this reduction size.)
  * Short-N warmup junk matmuls bridge the framework preamble to
    first-data so the HAM clock gate never re-throttles (v1 lost ~19us
    to a 9.3us PE idle + 1.2GHz cold restart mid-kernel).
  * fp16 output (host upcasts); fp16 operand numerics: end-to-end rel
    err ~4e-4 vs the 2e-2 gate.
  * Sharding: data-parallel over batch, 2 images per core on 8 cores.
    x2 passthrough is host-side.
"""
import math

import numpy as np

B, C, H, W = 16, 128, 64, 64
PDIM, SK, LK = 64, 3, 13
PAD = LK // 2  # 6
HP, WP = H + 2 * PAD, W + 2 * PAD  # 76, 76
NCORES = 8
BPC = B // NCORES  # images per core
NT = 85            # tap tiles: 78 horizontal pairs + 6 vertical pairs + 1 single
NCHUNK = 8         # 512-pixel chunks per image
CHUNK = H * W // NCHUNK  # 512
NWARM = 38         # junk N=128 matmuls bridging preamble -> first data

# tile t: t = i*6+p (p<6) -> horizontal pair ((i,2p),(i,2p+1));
#         t = 78+v -> vertical pair ((2v,12),(2v+1,12)); t = 84 -> single (12,12)
# central 3x3 dyn taps (i,j in 5..7): j=5 -> tile i*6+2 (B half), j=6/7 -> tile
# i*6+3 (A/B halves); 6 per-batch modified tiles.
_MOD_TILES = [5 * 6 + 2, 6 * 6 + 2, 7 * 6 + 2, 5 * 6 + 3, 6 * 6 + 3, 7 * 6 + 3]
_MOD_SLOT = {t: s for s, t in enumerate(_MOD_TILES)}

_ERF = np.vectorize(math.erf, otypes=[np.float64])

_CACHED_NC = None


def _build_nc():
    import concourse.mybir as mybir
    import concourse.tile as tile
    from concourse import bacc

    f32 = mybir.dt.float32
    f16 = mybir.dt.float16

    nc = bacc.Bacc(None, target_bir_lowering=False)
    la = nc.dram_tensor("la", [BPC, 128, HP, WP], f16, kind="ExternalInput")
    lb = nc.dram_tensor("lb", [BPC, 128, HP, WP], f16, kind="ExternalInput")
    wsh = nc.dram_tensor("wsh", [128, NT * 80], f16, kind="ExternalInput")
    wmod = nc.dram_tensor("wmod", [BPC, 128, 6 * 64], f16, kind="ExternalInput")
    y = nc.dram_tensor("y", [BPC, PDIM, H * W], f16, kind="ExternalOutput")

    NSH_FIRST = 28  # weight tiles shipped in the leading DMA slice

    with tile.TileContext(nc) as tc:
        with (
            tc.tile_pool(name="wpool", bufs=1) as wpool,
            tc.tile_pool(name="wmpool", bufs=2) as wmpool,
            tc.tile_pool(name="xpool", bufs=2) as xpool,
            tc.tile_pool(name="xbpool", bufs=2) as xbpool,
            tc.tile_pool(name="opool", bufs=3) as opool,
            tc.tile_pool(name="pspool", bufs=4, space="PSUM") as pspool,
        ):
            # input DMAs, in rough order of need; row-split so chunk-pair 0
            # can start after the first slice (tile deps are region-overlap)
            la_sb = [xpool.tile([128, HP, WP], f16, name=f"la{b}") for b in range(BPC)]
            lb_sb = [xbpool.tile([128, HP, WP], f16, name=f"lb{b}") for b in range(BPC)]
            wsh_sb = wpool.tile([128, NT * 80], f16)
            wm_sb = [wmpool.tile([128, 6 * 64], f16, name=f"wm{b}") for b in range(BPC)]

            # the single DMA queue drains roughly FIFO at ~360GB/s after a
            # ~2us posting ramp, so completion time ~= cumulative prefix
            # bytes; order by first use.  chunk-pair cp of tap row i reads
            # la rows i+16cp .. i+16cp+16.
            nc.sync.dma_start(out=la_sb[0][:, 0:16, :], in_=la[0, :, 0:16, :])
            nc.sync.dma_start(out=wsh_sb[:, 0:480], in_=wsh[:, 0:480])
            nc.sync.dma_start(out=la_sb[0][:, 16:24, :], in_=la[0, :, 16:24, :])
            nc.sync.dma_start(out=wsh_sb[:, 480:1600], in_=wsh[:, 480:1600])
            nc.sync.dma_start(out=wm_sb[0][:], in_=wmod[0])
            nc.sync.dma_start(out=wsh_sb[:, 1600:2720], in_=wsh[:, 1600:2720])
            nc.sync.dma_start(out=la_sb[0][:, 24:40, :], in_=la[0, :, 24:40, :])
            nc.sync.dma_start(out=wsh_sb[:, 2720:4160], in_=wsh[:, 2720:4160])
            nc.sync.dma_start(out=la_sb[0][:, 40:, :], in_=la[0, :, 40:, :])
            nc.sync.dma_start(out=wsh_sb[:, 4160:], in_=wsh[:, 4160:])
            nc.sync.dma_start(out=lb_sb[0][:], in_=lb[0])
            nc.sync.dma_start(out=la_sb[1][:], in_=la[1])
            nc.sync.dma_start(out=lb_sb[1][:], in_=lb[1])
            # wm1 rides the scalar hwdge queue: content isn't needed until
            # ~85us, and the post spins up that queue (~4.6us ramp) so the
            # final output DMA can use it concurrently with the sync queue
            nc.scalar.dma_start(out=wm_sb[1][:], in_=wmod[1])

            # PE warmup: junk matmuls on a zeroed scratch tile keep the PE busy
            # (HAM un-throttles 1.2 -> 2.4 GHz) while the input DMAs run.
            # short-N junk matmuls: fine-grained bridge (one MM ~107ns cold /
            # 56ns warm) so the in-order PE queue frees up right when the
            # first data lands
            scratch = wpool.tile([128, 128], f16)
            nc.vector.memset(scratch[:], 0.0)
            ps_warm = pspool.tile([128, 128], f32, name="ps_warm", bufs=1)
            for wi in range(NWARM):
                nc.tensor.matmul(
                    ps_warm[0:64, :],
                    lhsT=scratch[:, 0:64],
                    rhs=scratch[:, :],
                    start=(wi == 0),
                    stop=(wi == NWARM - 1),
                    tile_position=(0, 0),
                    skip_group_check=True,
                )

            for b in range(BPC):
                for cp in range(NCHUNK // 2):
                    ps = pspool.tile([128, CHUNK], f32)
                    for t in range(NT):
                        s = _MOD_SLOT.get(t)
                        w_ap = (
                            wm_sb[b][:, s * 64 : (s + 1) * 64]
                            if s is not None
                            else wsh_sb[:, t * 80 : t * 80 + 64]
                        )
                        for half in (0, 1):
                            R = 8 * (2 * cp + half)
                            if t < 78:
                                i, p = divmod(t, 6)
                                rhs = la_sb[b][:, i + R : i + R + 8, 2 * p : 2 * p + 64]
                            elif t < 84:
                                v = t - 78
                                rhs = lb_sb[b][:, 2 * v + R : 2 * v + R + 8, 12:76]
                            else:
                                rhs = la_sb[b][:, 12 + R : 12 + R + 8, 12:76]
                            nc.tensor.matmul(
                                ps[64 * half : 64 * (half + 1), :],
                                lhsT=w_ap,
                                rhs=rhs,
                                start=(t == 0),
                                stop=(t == NT - 1),
                                tile_position=(0, 64 * half),
                                skip_group_check=True,
                            )
                    ot = opool.tile([128, CHUNK], f16)
                    last = b == BPC - 1 and cp == NCHUNK // 2 - 1
                    if last:
                        # final drain: one cast (cost scales with free dim,
                        # not partitions), then ship the halves on BOTH hwdge
                        # queues so posts+transfers overlap
                        nc.vector.tensor_copy(ot[:], ps[:])
                        nc.sync.dma_start(
                            out=y[b, :, (2 * cp) * CHUNK : (2 * cp + 1) * CHUNK],
                            in_=ot[0:64, :],
                        )
                        nc.scalar.dma_start(
                            out=y[b, :, (2 * cp + 1) * CHUNK : (2 * cp + 2) * CHUNK],
                            in_=ot[64:128, :],
                        )
                    else:
                        # drain on the idle Scalar engine: its PSUM reads
                        # don't contend with the PE like DVE's do (DVE drain
                        # was slowing tiles 1-5 of the next chunk-pair)
                        nc.scalar.copy(ot[:], ps[:])
                        nc.sync.dma_start(
                            out=y[b, :, (2 * cp) * CHUNK : (2 * cp + 1) * CHUNK],
                            in_=ot[0:64, :],
                        )
                        nc.sync.dma_start(
                            out=y[b, :, (2 * cp + 1) * CHUNK : (2 * cp + 2) * CHUNK],
                            in_=ot[64:128, :],
                        )
    nc.compile()
    return nc


def _get_nc():
    global _CACHED_NC
    if _CACHED_NC is None:
        _CACHED_NC = _build_nc()
    return _CACHED_NC


def _host_dyn(x, w1, b1, w2, b2):
    """dwc_proj MLP on host, float64: dyn [B, 64, 9]."""
    pooled = x[:, :PDIM].mean(axis=(2, 3), dtype=np.float64)      # [B, 64]
    z = pooled @ w1.T.astype(np.float64) + b1.astype(np.float64)  # [B, 32]
    h = 0.5 * z * (1.0 + _ERF(z / math.sqrt(2.0)))                # exact gelu
    dyn = h @ w2.T.astype(np.float64) + b2.astype(np.float64)     # [B, 576]
    return dyn.reshape(B, PDIM, SK * SK)


def _host_weights(lk_filter, dyn):
    """Build shared supertiles + per-batch modified central supertiles.

    Tap tile t is [128, 64]: rows 0-63 = lk[o, c, tapA].T, rows 64-127 =
    tapB (lhsT layout [K=c, M=o]).  Supertile = the tile duplicated in
    both column halves -> [128, 128]."""
    lkT = lk_filter.transpose(1, 0, 2, 3).astype(np.float32)  # [c, o, i, j]
    Wt = np.zeros((NT, 128, 64), np.float32)
    for i in range(LK):
        for p in range(6):
            Wt[i * 6 + p, 0:64, :] = lkT[:, :, i, 2 * p]
            Wt[i * 6 + p, 64:128, :] = lkT[:, :, i, 2 * p + 1]
    for v in range(6):
        Wt[78 + v, 0:64, :] = lkT[:, :, 2 * v, 12]
        Wt[78 + v, 64:128, :] = lkT[:, :, 2 * v + 1, 12]
    Wt[84, 0:64, :] = lkT[:, :, 12, 12]

    ar = np.arange(64)
    Wmod = np.zeros((B, 6, 128, 64), np.float32)
    for ii, i in enumerate((5, 6, 7)):
        t2, t3 = i * 6 + 2, i * 6 + 3
        u = i - 5
        for b in range(B):
            m2 = Wt[t2].copy()
            m3 = Wt[t3].copy()
            m2[64 + ar, ar] += dyn[b, :, u * 3 + 0].astype(np.float32)  # tap (i,5)
            m3[ar, ar] += dyn[b, :, u * 3 + 1].astype(np.float32)       # tap (i,6)
            m3[64 + ar, ar] += dyn[b, :, u * 3 + 2].astype(np.float32)  # tap (i,7)
            Wmod[b, ii] = m2
            Wmod[b, 3 + ii] = m3

    Wp = np.zeros((NT, 128, 80), np.float32)   # 160B tile stride: rotate
    Wp[:, :, 0:64] = Wt                        # SBUF address phase per tile
    wsh_np = np.ascontiguousarray(
        Wp.transpose(1, 0, 2).reshape(128, NT * 80)
    ).astype(np.float16)
    wmod_np = np.ascontiguousarray(
        Wmod.transpose(0, 2, 1, 3).reshape(B, 128, 6 * 64)
    ).astype(np.float16)
    return wsh_np, wmod_np


def _host_layouts(x1_f16):
    """Padded SBUF layouts, host-built.  LA: partitions 0-63 image at
    (row+6, col+6), 64-127 shifted left one column (col+5).  LB: 0-63
    same, 64-127 shifted up one row (row+5)."""
    la = np.zeros((B, 128, HP, WP), np.float16)
    lb = np.zeros((B, 128, HP, WP), np.float16)
    la[:, 0:64, PAD : PAD + H, PAD : PAD + W] = x1_f16
    la[:, 64:128, PAD : PAD + H, PAD - 1 : PAD - 1 + W] = x1_f16
    lb[:, 0:64, PAD : PAD + H, PAD : PAD + W] = x1_f16
    lb[:, 64:128, PAD - 1 : PAD - 1 + H, PAD : PAD + W] = x1_f16
    return la, lb


def _prepare_in_maps(x, lk_filter, w1, b1, w2, b2):
    x = np.asarray(x, dtype=np.float32)
    dyn = _host_dyn(x, np.asarray(w1), np.asarray(b1), np.asarray(w2), np.asarray(b2))
    wsh_np, wmod_np = _host_weights(np.asarray(lk_filter, dtype=np.float32), dyn)
    x1_f16 = x[:, :PDIM].astype(np.float16)
    la, lb = _host_layouts(x1_f16)
    in_maps = []
    for k in range(NCORES):
        b0 = k * BPC
        in_maps.append(
            {
                "la": np.ascontiguousarray(la[b0 : b0 + BPC]),
                "lb": np.ascontiguousarray(lb[b0 : b0 + BPC]),
                "wsh": wsh_np,
                "wmod": np.ascontiguousarray(wmod_np[b0 : b0 + BPC]),
            }
        )
    return in_maps


def kernel(x, lk_filter, w1, b1, w2, b2):
    from concourse.bass_utils import run_bass_kernel_spmd

    x = np.asarray(x, dtype=np.float32)
    in_maps = _prepare_in_maps(x, lk_filter, w1, b1, w2, b2)
    nc = _get_nc()
    res = run_bass_kernel_spmd(nc, in_maps, core_ids=list(range(NCORES)))

    out = np.empty((B, C, H, W), np.float32)
    for k in range(NCORES):
        b0 = k * BPC
        out[b0 : b0 + BPC, :PDIM] = (
            res.results[k]["y"].astype(np.float32).reshape(BPC, PDIM, H, W)
        )
    out[:, PDIM:] = x[:, PDIM:]
    return out
